# revision 1
# baseline (speedup 1.0000x reference)
"""Trainium2 kernel for ChannelQuadLayer.

Per-pixel quadratic channel expansion + 1x1 conv:
    quad = x[:, ii] * x[:, jj]  (all 2080 upper-tri channel pairs)
    y    = concat([x, quad])    -> [B, 2144, H, W]
    out  = einsum('bchw,oc->bohw', y, fc_w)

Strategy (8 NeuronCores, batch-parallel, one sample per core):
  * The 2080 unordered channel pairs are exactly the cyclic diagonals
    d=0..32 of the 64-channel index ring: pairs {i, (i+d)%64}.
  * Host prepares 9 "rotation buffers" B_k = [roll(x,-t_k); roll(x,-u_k)]
    (128 partitions x 4096 pixels). A single elementwise multiply of two
    such buffers yields TWO complete cyclic diagonals (top half: diagonal
    t_j - t_i, bottom half: u_j - u_i). A difference cover (found by
    search) produces all diagonals 1..32 in 16 multiplies; diagonal 0
    (squares) comes from one ScalarE Square op. All vector ops use the
    full 128 partitions with 0-based alignment.
  * y-rows: 64 linear + 64 squares + 16*128 pair rows = 2176 = 17*128,
    an exact 17-chunk contraction. fc_w is permuted/padded to this row
    order on the host (duplicate pair rows get zero weight).
  * GEMM: out[256, 4096] = Wt[2176, 256]^T @ y[2176, 4096] on TensorE
    in float32r (full-rate fp32, ~2e-4 rel err), accumulating 17
    chunks into PSUM, k-outer so each y chunk is consumed right after
    its producer.
  * Pixel passes have variable width [512, 1024, 1024, 1024, 512]: the
    small first pass lets VectorE start early, the small last pass
    shortens the kernel tail.
"""

import sys

sys.path.insert(0, "/opt/trn_rl_repo")

import numpy as np

import concourse.bass as bass
import concourse.tile as tile
from concourse import bacc, mybir
from concourse.bass_utils import run_bass_kernel_spmd

B, C, H, W = 8, 64, 64, 64
PIX = H * W  # 4096
OUT = 256
NCORES = 8

# rotation difference cover: ops (i,j) give diagonals D(t_j-t_i) (top half)
# and D(u_j-u_i) (bottom half); together exactly {1..32}.
T_ROT = [0, 8, 22, 24, 42, 48, 49, 57, 60]
U_ROT = [0, 59, 16, 38, 55, 22, 30, 54, 35]
OPS = [(1, 3), (2, 3), (1, 4), (2, 4), (3, 4), (4, 5), (1, 6), (2, 6),
       (6, 7), (0, 7), (4, 7), (5, 7), (2, 8), (3, 8), (5, 8), (6, 8)]
NB = len(T_ROT)        # 9 rotation buffers
KCH = 1 + len(OPS)     # 17 contraction chunks of 128 rows
PASS_FD = [512, 1024, 1024, 1024, 512]
assert sum(PASS_FD) == PIX
NPASS = len(PASS_FD)

F32 = mybir.dt.float32
F32R = mybir.dt.float32r


def row_pairs():
    """Channel pair (c1, c2) for every global y row, or ('lin', c)."""
    rows = []
    for p in range(128):  # chunk 0
        rows.append(("lin", p) if p < 64 else (p - 64, p - 64))
    for (i, j) in OPS:
        for p in range(128):
            if p < 64:
                c1, c2 = (p + T_ROT[i]) % 64, (p + T_ROT[j]) % 64
            else:
                c1, c2 = (p - 64 + U_ROT[i]) % 64, (p - 64 + U_ROT[j]) % 64
            rows.append((min(c1, c2), max(c1, c2)))
    return rows


def build_wt(fc_w):
    """Permute fc_w [OUT, 2144] into Wt [KCH, 128, OUT] matching y rows."""
    ii, jj = np.triu_indices(C)
    pair2col = {(a, b): C + k for k, (a, b) in enumerate(zip(ii, jj))}
    wt = np.zeros((KCH * 128, OUT), np.float32)
    seen = set()
    for g, r in enumerate(row_pairs()):
        if r[0] == "lin":
            wt[g] = fc_w[:, r[1]]
        elif r not in seen:
            seen.add(r)
            wt[g] = fc_w[:, pair2col[r]]
    assert len(seen) == C * (C + 1) // 2
    return np.ascontiguousarray(wt.reshape(KCH, 128, OUT))


_cached = None


def _build_module():
    global _cached
    if _cached is not None:
        return _cached
    nc = bacc.Bacc("TRN2", target_bir_lowering=False, debug=False,
                   num_devices=NCORES)
    b_d = [nc.dram_tensor(f"b{i}", [128, PIX], F32R, kind="ExternalInput")
           for i in range(NB)]
    # weight matrix, partition-major so DMA rows are 17KB contiguous
    wt_d = nc.dram_tensor("wt", [128, KCH * OUT], F32R, kind="ExternalInput")
    out_d = nc.dram_tensor("out", [2, 128, PIX], F32, kind="ExternalOutput")

    with tile.TileContext(nc) as tc:
        with tc.tile_pool(name="wt", bufs=1) as wt_pool, \
             tc.tile_pool(name="bsrc", bufs=2) as b_pool, \
             tc.tile_pool(name="y", bufs=8) as y_pool, \
             tc.tile_pool(name="ostage", bufs=4) as o_pool, \
             tc.tile_pool(name="psum", bufs=8, space="PSUM") as ps_pool:

            wt_t = wt_pool.tile([128, KCH * OUT], F32R, name="wtt")
            WSPLIT = 9 * OUT  # chunks 0-8 in the first weight transfer

            off = 0
            for ps, FD in enumerate(PASS_FD):
                NT = max(1, FD // 512)
                NW = min(512, FD)  # matmul free width
                bt = []
                for i in range(NB):
                    t = b_pool.tile([128, 1024], F32R, tag=f"b{i}",
                                    name=f"b{i}_{ps}")
                    nc.sync.dma_start(t[:, :FD], b_d[i].ap()[:, off:off + FD])
                    bt.append(t)
                    if ps == 0 and i == 3:
                        nc.sync.dma_start(wt_t[:, :WSPLIT],
                                          wt_d.ap()[:, :WSPLIT])
                    if ps == 0 and i == 7:
                        nc.sync.dma_start(wt_t[:, WSPLIT:],
                                          wt_d.ap()[:, WSPLIT:])

                psum = [ps_pool.tile([128, 512], F32, tag="ps",
                                     name=f"ps{ps}_{g}")
                        for g in range(2 * NT)]

                for k in range(KCH):
                    yk = y_pool.tile([128, 1024], F32R, tag="y",
                                     name=f"y{ps}_{k}")
                    if k == 0:
                        # linear rows + squares, both from the resident b0 tile
                        nc.scalar.activation(
                            yk[0:64, :FD], bt[0][0:64, :FD],
                            mybir.ActivationFunctionType.Identity)
                        nc.scalar.activation(
                            yk[64:128, :FD], bt[0][64:128, :FD],
                            mybir.ActivationFunctionType.Square)
                    else:
                        i, j = OPS[k - 1]
                        nc.vector.tensor_mul(yk[:, :FD], bt[i][:, :FD],
                                             bt[j][:, :FD])
                    for m in range(2):
                        lhsT = wt_t[:, k * OUT + m * 128:k * OUT + (m + 1) * 128]
                        for n in range(NT):
                            nc.tensor.matmul(
                                psum[m * NT + n][:, :NW],
                                lhsT,
                                yk[:, n * NW:(n + 1) * NW],
                                start=(k == 0), stop=(k == KCH - 1))

                last = ps == NPASS - 1
                for m in range(2):
                    ot = o_pool.tile([128, 1024], F32, tag="ostage",
                                     name=f"o{ps}_{m}")
                    for n in range(NT):
                        src = psum[m * NT + n][:, :NW]
                        dst = ot[:, n * NW:(n + 1) * NW]
                        if last and m == 1:
                            # tail: drain half the PSUM on the idle VectorE
                            nc.vector.tensor_copy(dst, src)
                        else:
                            nc.scalar.activation(
                                dst, src, mybir.ActivationFunctionType.Identity)
                    eng = nc.sync if (last and m == 1) else nc.scalar
                    eng.dma_start(out_d.ap()[m, :, off:off + FD], ot[:, :FD])
                off += FD
    nc.compile()
    _cached = nc
    return nc


def make_in_maps(x, wt):
    # [KCH, 128, OUT] -> [128, KCH*OUT]
    wtp = np.ascontiguousarray(wt.transpose(1, 0, 2).reshape(128, KCH * OUT))
    in_maps = []
    for b in range(B):
        xc = np.ascontiguousarray(np.asarray(x[b], np.float32).reshape(C, PIX))
        m = {"wt": wtp}
        for i in range(NB):
            m[f"b{i}"] = np.ascontiguousarray(np.concatenate(
                [np.roll(xc, -T_ROT[i], axis=0), np.roll(xc, -U_ROT[i], axis=0)]))
        in_maps.append(m)
    return in_maps


def assemble_out(res):
    outs = []
    for b in range(B):
        o = res.results[b]["out"]  # [2, 128, PIX]
        outs.append(o.reshape(OUT, H, W))
    return np.stack(outs)


def kernel(x, fc_w):
    x = np.asarray(x, dtype=np.float32)
    fc_w = np.asarray(fc_w, dtype=np.float32)
    nc = _build_module()
    wt = build_wt(fc_w)
    res = run_bass_kernel_spmd(nc, make_in_maps(x, wt), list(range(NCORES)))
    return assemble_out(res)



# revision 2
# speedup vs baseline: 1.2872x; 1.2872x over previous
"""Trainium2 kernel for ChannelQuadLayer.

Per-pixel quadratic channel expansion + 1x1 conv:
    quad = x[:, ii] * x[:, jj]  (all 2080 upper-tri channel pairs)
    y    = concat([x, quad])    -> [B, 2144, H, W]
    out  = einsum('bchw,oc->bohw', y, fc_w)

Strategy (8 NeuronCores, batch-parallel, one sample per core):
  * The 2080 unordered channel pairs are exactly the cyclic diagonals
    d=0..32 of the 64-channel index ring: pairs {i, (i+d)%64}.
  * Host prepares 9 "rotation buffers" B_k = [roll(x,-t_k); roll(x,-u_k)]
    (128 partitions x 4096 pixels, bf16). One elementwise multiply of two
    such buffers yields TWO complete cyclic diagonals; a difference cover
    produces all diagonals 1..32 in 16 multiplies. Diagonal 0 (squares)
    and the linear rows come from buffer 0 (copy + self-multiply).
  * y-rows: 64 linear + 64 squares + 16*128 pair rows = 2176 = 17*128,
    an exact 17-chunk contraction. fc_w is permuted/padded to this row
    order on the host (duplicate pair rows get zero weight), cast bf16.
  * GEMM: out[256, 4096] = Wt[2176, 256]^T @ y[2176, 4096] on TensorE in
    bf16 (same PE rate as fp32r, half the SBUF/HBM traffic), accumulating
    17 chunks into fp32 PSUM, k-outer so each y chunk is consumed right
    after its producer. 4 pixel passes of 1024.
  * Engine split keeps the PE stream gap-free: VectorE produces ALL y
    chunks (incl. chunk 0 via copy+self-mul), ScalarE only drains PSUM
    and triggers output DMA, SP/GPSIMD queues carry the two grouped
    input streams. Pass 0 uses per-buffer DMAs ordered so the first
    chunks' inputs land first; the first weight slice is just chunk 0
    (64KB) so the PE starts ~2us after DMA begins.
"""

import sys

sys.path.insert(0, "/opt/trn_rl_repo")

import numpy as np
import ml_dtypes

import concourse.bass as bass
import concourse.tile as tile
from concourse import bacc, mybir
from concourse.bass_utils import run_bass_kernel_spmd

B, C, H, W = 8, 64, 64, 64
PIX = H * W  # 4096
OUT = 256
NCORES = 8

# rotation difference cover: ops (i,j) give diagonals D(t_j-t_i) (top half)
# and D(u_j-u_i) (bottom half); together exactly {1..32}.
T_ROT = [0, 8, 22, 24, 42, 48, 49, 57, 60]
U_ROT = [0, 59, 16, 38, 55, 22, 30, 54, 35]
# group A = buffers 0-4, group B = buffers 5-8; A-only ops first so pass 0
# can start computing while group B is still in flight.
OPS_A = [(1, 3), (2, 3), (1, 4), (2, 4), (3, 4)]
OPS_B = [(4, 5), (1, 6), (2, 6), (6, 7), (0, 7), (4, 7),
         (5, 7), (2, 8), (3, 8), (5, 8), (6, 8)]
OPS = OPS_A + OPS_B
NB = len(T_ROT)        # 9 rotation buffers
NA, NBB = 5, 4         # buffers per group
KCH = 1 + len(OPS)     # 17 contraction chunks of 128 rows
NPASS = 4
FD = PIX // NPASS      # 1024 pixels per pass
NT = 2
NW = FD // NT          # 512 matmul free width (one PSUM bank)

F32 = mybir.dt.float32
BF16 = mybir.dt.bfloat16
NPBF16 = ml_dtypes.bfloat16


def row_pairs():
    """Channel pair (c1, c2) for every global y row, or ('lin', c)."""
    rows = []
    for p in range(128):  # chunk 0
        rows.append(("lin", p) if p < 64 else (p - 64, p - 64))
    for (i, j) in OPS:
        for p in range(128):
            if p < 64:
                c1, c2 = (p + T_ROT[i]) % 64, (p + T_ROT[j]) % 64
            else:
                c1, c2 = (p - 64 + U_ROT[i]) % 64, (p - 64 + U_ROT[j]) % 64
            rows.append((min(c1, c2), max(c1, c2)))
    return rows


def build_wt(fc_w):
    """Permute fc_w [OUT, 2144] into Wt [KCH, 128, OUT] matching y rows."""
    ii, jj = np.triu_indices(C)
    pair2col = {(a, b): C + k for k, (a, b) in enumerate(zip(ii, jj))}
    wt = np.zeros((KCH * 128, OUT), np.float32)
    seen = set()
    for g, r in enumerate(row_pairs()):
        if r[0] == "lin":
            wt[g] = fc_w[:, r[1]]
        elif r not in seen:
            seen.add(r)
            wt[g] = fc_w[:, pair2col[r]]
    assert len(seen) == C * (C + 1) // 2
    return np.ascontiguousarray(wt.reshape(KCH, 128, OUT))


_cached = None


def _build_module():
    global _cached
    if _cached is not None:
        return _cached
    nc = bacc.Bacc("TRN2", target_bir_lowering=False, debug=False,
                   num_devices=NCORES)
    # grouped rotation buffers, pass-major so each pass is one contiguous DMA
    ba_d = nc.dram_tensor("ba", [128, NPASS * NA * FD], BF16,
                          kind="ExternalInput")
    bb_d = nc.dram_tensor("bb", [128, NPASS * NBB * FD], BF16,
                          kind="ExternalInput")
    # weight matrix, partition-major so DMA rows are contiguous
    wt_d = nc.dram_tensor("wt", [128, KCH * OUT], BF16, kind="ExternalInput")
    out_d = nc.dram_tensor("out", [2, 128, PIX], BF16, kind="ExternalOutput")

    with tile.TileContext(nc) as tc:
        with tc.tile_pool(name="wt", bufs=1) as wt_pool, \
             tc.tile_pool(name="ba", bufs=3) as ba_pool, \
             tc.tile_pool(name="bb", bufs=3) as bb_pool, \
             tc.tile_pool(name="y", bufs=8) as y_pool, \
             tc.tile_pool(name="ostage", bufs=4) as o_pool, \
             tc.tile_pool(name="psum", bufs=8, space="PSUM") as ps_pool:


            wt_t = wt_pool.tile([128, KCH * OUT], BF16, name="wtt")

            drains = [None] * NPASS

            def emit_drain(ps, psum, off):
                last = ps == NPASS - 1
                for m in range(2):
                    ot = o_pool.tile([128, FD], BF16, tag="ostage",
                                     name=f"o{ps}_{m}")
                    for n in range(NT):
                        src = psum[m * NT + n][:, :NW]
                        dst = ot[:, n * NW:(n + 1) * NW]
                        if last and m == 1:
                            # tail: drain half the PSUM on the idle VectorE
                            nc.vector.tensor_copy(dst, src)
                        else:
                            nc.scalar.activation(
                                dst, src, mybir.ActivationFunctionType.Identity)
                    eng = nc.sync if (last and m == 1) else nc.scalar
                    eng.dma_start(out_d.ap()[m, :, off:off + FD], ot[:, :FD])

            for ps in range(NPASS):
                off = ps * FD
                ba_t = ba_pool.tile([128, NA * FD], BF16, tag="ba",
                                    name=f"ba{ps}")
                bb_t = bb_pool.tile([128, NBB * FD], BF16, tag="bb",
                                    name=f"bb{ps}")
                a0 = ps * NA * FD
                b0 = ps * NBB * FD
                if ps == 0:
                    # weight chunk 0 first (64KB) so the PE can start early,
                    # then per-buffer slices in compute order.
                    nc.sync.dma_start(wt_t[:, :OUT], wt_d.ap()[:, :OUT])
                    for i in (0, 1, 3):
                        nc.sync.dma_start(
                            ba_t[:, i * FD:(i + 1) * FD],
                            ba_d.ap()[:, a0 + i * FD:a0 + (i + 1) * FD])
                    nc.sync.dma_start(wt_t[:, OUT:(1 + len(OPS_A)) * OUT],
                                      wt_d.ap()[:, OUT:(1 + len(OPS_A)) * OUT])
                    for i in (2, 4):
                        nc.sync.dma_start(
                            ba_t[:, i * FD:(i + 1) * FD],
                            ba_d.ap()[:, a0 + i * FD:a0 + (i + 1) * FD])
                    nc.sync.dma_start(wt_t[:, (1 + len(OPS_A)) * OUT:],
                                      wt_d.ap()[:, (1 + len(OPS_A)) * OUT:])
                    for i in range(NBB):
                        nc.gpsimd.dma_start(
                            bb_t[:, i * FD:(i + 1) * FD],
                            bb_d.ap()[:, b0 + i * FD:b0 + (i + 1) * FD])
                else:
                    nc.sync.dma_start(ba_t, ba_d.ap()[:, a0:a0 + NA * FD])
                    nc.gpsimd.dma_start(bb_t, bb_d.ap()[:, b0:b0 + NBB * FD])

                psum = [ps_pool.tile([128, NW], F32, tag="ps",
                                     name=f"ps{ps}_{g}")
                        for g in range(2 * NT)]

                for k in range(KCH):
                    yk = y_pool.tile([128, FD], BF16, tag="y",
                                     name=f"y{ps}_{k}")
                    if k == 0:
                        # linear rows + squares, both from the resident b0
                        nc.vector.tensor_copy(yk[0:64, :], ba_t[0:64, :FD])
                        nc.vector.tensor_mul(yk[64:128, :], ba_t[64:128, :FD],
                                             ba_t[64:128, :FD])
                    else:
                        i, j = OPS[k - 1]
                        src_i = ba_t[:, i * FD:(i + 1) * FD] if i < NA \
                            else bb_t[:, (i - NA) * FD:(i - NA + 1) * FD]
                        src_j = ba_t[:, j * FD:(j + 1) * FD] if j < NA \
                            else bb_t[:, (j - NA) * FD:(j - NA + 1) * FD]
                        nc.vector.tensor_mul(yk, src_i, src_j)
                    for m in range(2):
                        lhsT = wt_t[:, k * OUT + m * 128:k * OUT + (m + 1) * 128]
                        for n in range(NT):
                            nc.tensor.matmul(
                                psum[m * NT + n][:, :NW],
                                lhsT,
                                yk[:, n * NW:(n + 1) * NW],
                                start=(k == 0), stop=(k == KCH - 1))

                emit_drain(ps, psum, off)
    nc.compile()
    _cached = nc
    return nc


def make_in_maps(x, wt):
    # [KCH, 128, OUT] -> [128, KCH*OUT]
    wtp = np.ascontiguousarray(
        wt.transpose(1, 0, 2).reshape(128, KCH * OUT).astype(NPBF16))
    in_maps = []
    for b in range(B):
        xc = np.asarray(x[b], np.float32).reshape(C, PIX).astype(NPBF16)
        bufs = [np.concatenate([np.roll(xc, -t, axis=0),
                                np.roll(xc, -u, axis=0)])
                for t, u in zip(T_ROT, U_ROT)]
        # [128, NPASS, buf, FD] pass-major packing per group
        ba = np.stack([b_.reshape(128, NPASS, FD) for b_ in bufs[:NA]], axis=2)
        bb = np.stack([b_.reshape(128, NPASS, FD) for b_ in bufs[NA:]], axis=2)
        in_maps.append({
            "wt": wtp,
            "ba": np.ascontiguousarray(ba.reshape(128, NPASS * NA * FD)),
            "bb": np.ascontiguousarray(bb.reshape(128, NPASS * NBB * FD)),
        })
    return in_maps


def assemble_out(res):
    outs = []
    for b in range(B):
        o = np.asarray(res.results[b]["out"]).astype(np.float32)
        outs.append(o.reshape(OUT, H, W))
    return np.stack(outs)


def kernel(x, fc_w):
    x = np.asarray(x, dtype=np.float32)
    fc_w = np.asarray(fc_w, dtype=np.float32)
    nc = _build_module()
    wt = build_wt(fc_w)
    res = run_bass_kernel_spmd(nc, make_in_maps(x, wt), list(range(NCORES)))
    return assemble_out(res)
